# revision 7
# baseline (speedup 1.0000x reference)
# Dissipation network Bass kernel for TRN2, v6: v5 + L4 exp order swap (s3 before xs3); v3 + gate-chain reorder (DVE runs ahead of PE).
# vs v2: (a) pipeline interleave — body(ST i) is emitted between L2(i-1) and
# L3(i-1) so the ACT engine (the bottleneck) always has ready work queued;
# (b) fp16 input transposes (half PE stream cost, 1-bank PSUM tiles).
import numpy as np
import concourse.bass as bass
from concourse import bacc
import concourse.hw_specs as hw_specs
import concourse.bacc as bacc_mod
import concourse.mybir as mybir
import concourse.tile as tile

dt = mybir.dt
AF = mybir.ActivationFunctionType
ALU = mybir.AluOpType

_orig_get_tables = hw_specs.get_activation_tables


def _pinned_tables(arch):
    t = _orig_get_tables(arch)
    out = {}
    for name, fns in t.items():
        if name != "natural_log_exp_and_others":
            fns = fns - {AF.Exp, AF.Ln}
        out[name] = fns
    return out


bacc_mod.get_activation_tables = _pinned_tables

D, H = 16, 50
F = 1024
STB = 2 * F
NCHUNK = F // 512
NA = 8
OUTB = 8

W_SPECS = [
    ("W_xl1", D, H, 32), ("W_xin", D, H, 32), ("W_clinm", D, D, 32),
    ("W_clin", D, H, 0),
    ("W_xl2", H, H, 0), ("W_cp1m", H, H, 0), ("W_cl1m", H, D, 0),
    ("W_xp1", H, H, 0), ("W_cp1", H, H, 0), ("W_cl1", D, H, 0),
    ("W_xl3", H, H, 0), ("W_cp2m", H, H, 0), ("W_cl2m", H, D, 0),
    ("W_xp2", H, H, 0), ("W_cp2", H, H, 0), ("W_cl2", D, H, 0),
    ("W_xlo", H, 1, 0), ("W_cpom", H, H, 0), ("W_clom", H, D, 0),
    ("W_cpo", H, 1, 0), ("W_clo", D, 1, 0),
]
W_INFO = {}
_off = 0
for _n, _k, _m, _rb in W_SPECS:
    W_INFO[_n] = (_off, 64 + _rb + _k, 64 + _m, _k, _m, _rb)
    _off += 64 + _m
NW = _off

B_SPECS = ["b_xl1", "b_xin", "b_clinm", "b_xl2", "b_cp1m", "b_cl1m", "b_xp1",
           "b_xl3", "b_cp2m", "b_cl2m", "b_xp2", "b_xlo", "b_cpom", "b_clom"]
B_COL = {n: i for i, n in enumerate(B_SPECS)}
NB = len(B_SPECS)


def pack_weights(inputs):
    wpack = np.zeros((128, NW), dtype=np.float16)
    for n, k, m, rb in W_SPECS:
        wt = np.asarray(inputs[n]).astype(np.float32).T
        assert wt.shape == (k, m), (n, wt.shape)
        wh = wt.astype(np.float16)
        off = W_INFO[n][0]
        wpack[rb:rb + k, off:off + m] = wh
        wpack[64 + rb:64 + rb + k, off + 64:off + 64 + m] = wh
    bpack = np.zeros((128, NB), dtype=np.float32)
    for n in B_SPECS:
        b = np.asarray(inputs[n]).astype(np.float32)
        c = B_COL[n]
        if n == "b_xlo":
            bpack[:, c] = b[0]
        else:
            bpack[0:len(b), c] = b
            bpack[64:64 + len(b), c] = b
    ident = np.eye(128, dtype=np.float16)
    return wpack, bpack, ident


def build_program(n_rows):
    assert n_rows % (STB * OUTB) == 0
    nst = n_rows // STB
    nc = bacc.Bacc("TRN2", target_bir_lowering=False, debug=False,
                   enable_asserts=False)
    inp_d = nc.dram_tensor("input", [n_rows, 32], dt.float32, kind="ExternalInput")
    w_d = nc.dram_tensor("wpack", [128, NW], dt.float16, kind="ExternalInput")
    b_d = nc.dram_tensor("bpack", [128, NB], dt.float32, kind="ExternalInput")
    c_d = nc.dram_tensor("ident", [128, 128], dt.float16, kind="ExternalInput")
    stage_d = nc.dram_tensor("stage", [n_rows], dt.float32, kind="Internal")
    out_d = nc.dram_tensor("out", [n_rows, 1], dt.float32, kind="ExternalOutput")

    with tile.TileContext(nc) as tc:
        with tc.tile_pool(name="const", bufs=1) as cpool, \
             tc.tile_pool(name="inp", bufs=3) as inpool, \
             tc.tile_pool(name="x0p", bufs=3) as x0pool, \
             tc.tile_pool(name="mh", bufs=6) as mhpool, \
             tc.tile_pool(name="g", bufs=4) as gpool, \
             tc.tile_pool(name="stg", bufs=5) as stgpool, \
             tc.tile_pool(name="axs", bufs=6) as xspool, \
             tc.tile_pool(name="aout", bufs=3) as outpool, \
             tc.tile_pool(name="fin", bufs=2) as finpool, \
             tc.tile_pool(name="ps", bufs=3, space="PSUM") as ps, \
             tc.tile_pool(name="po", bufs=1, space="PSUM") as po:

            wt = cpool.tile([128, NW], dt.float16)
            nc.sync.dma_start(out=wt[:], in_=w_d.ap())
            bt = cpool.tile([128, NB], dt.float32)
            nc.sync.dma_start(out=bt[:], in_=b_d.ap())
            ct = cpool.tile([128, 128], dt.float16)
            nc.sync.dma_start(out=ct[:], in_=c_d.ap())

            def mm(psum_t, wname, rhs_t, start, stop):
                for c in range(NCHUNK):
                    mm_c(psum_t, wname, rhs_t, start, stop, c)

            def mm_c(psum_t, wname, rhs_t, start, stop, c):
                off, keff, meff, k, m, rb = W_INFO[wname]
                cs = slice(512 * c, 512 * (c + 1))
                nc.tensor.matmul(psum_t[0:meff, cs], wt[0:keff, off:off + meff],
                                 rhs_t[0:keff, cs], start=start, stop=stop)

            def exp_into(stg, half, psum_t, rows, bias_name):
                nc.scalar.activation(stg[0:rows, F * half:F * (half + 1)],
                                     psum_t[0:rows, 0:F], AF.Exp,
                                     bias=bt[0:rows, B_COL[bias_name]:B_COL[bias_name] + 1])

            def ln_pair(stg, rows):
                res = xspool.tile([rows, 2 * F], dt.float16, tag="lnp")
                nc.scalar.activation(res[0:rows, :], stg[0:rows, :], AF.Ln, bias=1.0)
                return res[0:rows, 0:F], res[0:rows, F:2 * F]

            def softplus1(psum_t, rows, bias_name):
                stg = stgpool.tile([rows, F], dt.float32, tag="stg1")
                nc.scalar.activation(stg[0:rows, :], psum_t[0:rows, 0:F], AF.Exp,
                                     bias=bt[0:rows, B_COL[bias_name]:B_COL[bias_name] + 1])
                res = xspool.tile([rows, F], dt.float16, tag="sp1")
                nc.scalar.activation(res[0:rows, :], stg[0:rows, :], AF.Ln, bias=1.0)
                return res

            def make_h(c, bcl, dh_p):
                x0a = c["x0a"]
                h = mhpool.tile([80, F], dt.float16, tag="mh")
                for cc in range(NCHUNK):
                    cs = slice(512 * cc, 512 * (cc + 1))
                    nc.vector.scalar_tensor_tensor(
                        h[0:80, cs], dh_p[0:80, cs],
                        bt[0:80, B_COL[bcl]:B_COL[bcl] + 1],
                        x0a[0:80, cs], op0=ALU.add, op1=ALU.mult)
                c["h_pending"] = h

            def gate_tail(c, xs_p, cl_w, cp_w, bcl, dh_p, axs, asv):
                x0a = c["x0a"]
                h = c.pop("h_pending")
                g = gpool.tile([114, F], dt.float16, tag="g")
                for cc in range(NCHUNK):
                    mm_c(xs_p, cl_w, h, False, False, cc)
                for cc in range(NCHUNK):
                    cs = slice(512 * cc, 512 * (cc + 1))
                    nc.vector.tensor_tensor(g[0:114, cs], axs[0:114, cs],
                                            asv[0:114, cs], op=ALU.mult)
                for cc in range(NCHUNK):
                    mm_c(xs_p, cp_w, g, False, cc == NCHUNK - 1, cc)

            def body(st):
                r0 = st * STB
                in_t = inpool.tile([128, NA * 112], dt.float32, tag="int")
                for pg in range(2):
                    rb = r0 + pg * F
                    src_x = inp_d.ap()[rb:rb + F, 0:16].rearrange("(a p) f -> p a f", p=128)
                    src_s = inp_d.ap()[rb:rb + F, 16:32].rearrange("(a p) f -> p a f", p=128)
                    r3 = in_t[:].rearrange("p (a q) -> p a q", q=112)
                    nc.sync.dma_start(out=r3[:, :, 64 * pg + 32:64 * pg + 48], in_=src_x)
                    nc.sync.dma_start(out=r3[:, :, 64 * pg:64 * pg + 16], in_=src_s)
                for jr in (16, 48, 80):
                    nc.gpsimd.memset(r3[:, :, jr:jr + 16], 0.0)
                in16 = inpool.tile([128, NA * 112], dt.float16, tag="in16")
                nc.vector.tensor_copy(in16[:], in_t[:])
                pT = ps.tile([112, F], dt.float16, tag="ps")
                for a in range(NA):
                    nc.tensor.transpose(pT[0:112, 128 * a:128 * (a + 1)],
                                        in16[:, 112 * a:112 * a + 112], ct[:])
                x0a = x0pool.tile([112, F], dt.float16, tag="x0a")
                nc.vector.tensor_copy(x0a[0:112, :], pT[0:112, :])

                # ---- L1 ----
                p_dm = ps.tile([80, F], dt.float32, tag="ps")
                mm(p_dm, "W_clinm", x0a, True, True)
                m1 = mhpool.tile([80, F], dt.float16, tag="mh")
                for cc in range(NCHUNK):
                    cs = slice(512 * cc, 512 * (cc + 1))
                    nc.vector.scalar_tensor_tensor(
                        m1[0:80, cs], p_dm[0:80, cs],
                        bt[0:80, B_COL["b_clinm"]:B_COL["b_clinm"] + 1],
                        x0a[0:80, cs], op0=ALU.add, op1=ALU.mult)
                p_x1 = ps.tile([114, F], dt.float32, tag="ps")
                mm(p_x1, "W_xin", x0a, True, True)
                stgA = stgpool.tile([114, 2 * F], dt.float32, tag="stgw")
                exp_into(stgA, 0, p_x1, 114, "b_xin")
                p_xs1 = ps.tile([114, F], dt.float32, tag="ps")
                mm(p_xs1, "W_xl1", x0a, True, False)
                for cc in range(NCHUNK):
                    mm_c(p_xs1, "W_clin", m1, False, cc == NCHUNK - 1, cc)
                exp_into(stgA, 1, p_xs1, 114, "b_xl1")
                a_x1, a_xs1 = ln_pair(stgA, 114)
                return dict(r0=r0, st=st, x0a=x0a, a_x1=a_x1, a_xs1=a_xs1)

            def stage_L2(c):
                a_x1, a_xs1 = c["a_x1"], c["a_xs1"]
                p_dh1 = ps.tile([80, F], dt.float32, tag="ps")
                mm(p_dh1, "W_cl1m", a_x1, True, True)
                make_h(c, "b_cl1m", p_dh1)
                p_x2 = ps.tile([114, F], dt.float32, tag="ps")
                mm(p_x2, "W_xp1", a_x1, True, True)
                stgB = stgpool.tile([114, 2 * F], dt.float32, tag="stgw")
                exp_into(stgB, 0, p_x2, 114, "b_xp1")
                p_s1 = ps.tile([114, F], dt.float32, tag="ps")
                mm(p_s1, "W_cp1m", a_x1, True, True)
                exp_into(stgB, 1, p_s1, 114, "b_cp1m")
                a_x2, a_s1 = ln_pair(stgB, 114)
                p_xs2 = ps.tile([114, F], dt.float32, tag="ps")
                mm(p_xs2, "W_xl2", a_x1, True, False)
                gate_tail(c, p_xs2, "W_cl1", "W_cp1", "b_cl1m", p_dh1, a_xs1, a_s1)
                a_xs2 = softplus1(p_xs2, 114, "b_xl2")
                c.update(a_x2=a_x2, a_s1=a_s1, a_xs2=a_xs2)

            def stage_L3(c):
                a_x2, a_xs2 = c["a_x2"], c["a_xs2"]
                p_dh2 = ps.tile([80, F], dt.float32, tag="ps")
                mm(p_dh2, "W_cl2m", a_x2, True, True)
                make_h(c, "b_cl2m", p_dh2)
                p_x3 = ps.tile([114, F], dt.float32, tag="ps")
                mm(p_x3, "W_xp2", a_x2, True, True)
                stgC = stgpool.tile([114, 2 * F], dt.float32, tag="stgw")
                exp_into(stgC, 0, p_x3, 114, "b_xp2")
                p_s2 = ps.tile([114, F], dt.float32, tag="ps")
                mm(p_s2, "W_cp2m", a_x2, True, True)
                exp_into(stgC, 1, p_s2, 114, "b_cp2m")
                a_x3, a_s2 = ln_pair(stgC, 114)
                p_xs3 = ps.tile([114, F], dt.float32, tag="ps")
                mm(p_xs3, "W_xl3", a_x2, True, False)
                gate_tail(c, p_xs3, "W_cl2", "W_cp2", "b_cl2m", p_dh2, a_xs2, a_s2)
                c.update(a_x3=a_x3, a_s2=a_s2, p_xs3=p_xs3)

            def stage_L4(c):
                a_x3, a_xs2, p_xs3 = c["a_x3"], c["a_xs2"], c["p_xs3"]
                stgD = stgpool.tile([114, 2 * F], dt.float32, tag="stgw")
                p_dh3 = ps.tile([80, F], dt.float32, tag="ps")
                mm(p_dh3, "W_clom", a_x3, True, True)
                make_h(c, "b_clom", p_dh3)
                p_s3 = ps.tile([114, F], dt.float32, tag="ps")
                mm(p_s3, "W_cpom", a_x3, True, True)
                exp_into(stgD, 1, p_s3, 114, "b_cpom")
                exp_into(stgD, 0, p_xs3, 114, "b_xl3")
                a_xs3, a_s3 = ln_pair(stgD, 114)
                p_out = po.tile([65, F], dt.float32, tag="po")
                mm(p_out, "W_xlo", a_x3, True, False)
                gate_tail(c, p_out, "W_clo", "W_cpo", "b_clom", p_dh3, a_xs3, a_s3)
                c["p_out"] = p_out

            def tail(c):
                r0, p_out = c["r0"], c["p_out"]
                o2 = outpool.tile([65, F], dt.float32, tag="o2")
                nc.vector.tensor_copy(o2[0:65, :], p_out[0:65, :])
                nc.sync.dma_start(out=stage_d.ap()[r0:r0 + F], in_=o2[0:1, :])
                nc.sync.dma_start(out=stage_d.ap()[r0 + F:r0 + STB], in_=o2[64:65, :])

            def finalize(b):
                rows = OUTB * STB
                r0 = b * rows
                ft = finpool.tile([128, rows // 128], dt.float32, tag="fin")
                nc.sync.dma_start(
                    out=ft[:],
                    in_=stage_d.ap()[r0:r0 + rows].rearrange("(p f) -> p f", p=128))
                fe = finpool.tile([128, rows // 128], dt.float32, tag="fine")
                nc.scalar.activation(fe[:], ft[:], AF.Exp,
                                     bias=bt[:, B_COL["b_xlo"]:B_COL["b_xlo"] + 1])
                nc.scalar.activation(ft[:], fe[:], AF.Ln, bias=1.0)
                nc.sync.dma_start(out=out_d.ap()[r0:r0 + rows, 0:1], in_=ft[:])

            # software pipeline: body(i) | L2(i-1) interleaved, then L3/L4/tail
            prev = None
            done = 0
            for st in range(nst):
                if prev is not None:
                    stage_L2(prev)
                cur = body(st)
                if prev is not None:
                    stage_L3(prev)
                    stage_L4(prev)
                    tail(prev)
                    done += 1
                    if done % OUTB == 0:
                        finalize(done // OUTB - 1)
                prev = cur
            stage_L2(prev)
            stage_L3(prev)
            stage_L4(prev)
            tail(prev)
            done += 1
            if done % OUTB == 0:
                finalize(done // OUTB - 1)

    nc.finalize()
    return nc


# ---------------------------------------------------------------------------
N_CORES = 8
_program_cache = {}


def _get_program(core_rows):
    if core_rows not in _program_cache:
        _program_cache[core_rows] = build_program(core_rows)
    return _program_cache[core_rows]


def kernel(**inputs):
    from concourse.bass_utils import run_bass_kernel_spmd
    x = np.ascontiguousarray(np.asarray(inputs["input"], dtype=np.float32))
    B = x.shape[0]
    assert x.shape[1] == 2 * D
    core_rows = B // N_CORES
    assert core_rows * N_CORES == B and core_rows % (STB * OUTB) == 0, (B,)
    wpack, bpack, ident = pack_weights(inputs)
    nc = _get_program(core_rows)
    in_maps = [{
        "input": x[i * core_rows:(i + 1) * core_rows],
        "wpack": wpack, "bpack": bpack, "ident": ident,
    } for i in range(N_CORES)]
    res = run_bass_kernel_spmd(nc, in_maps, list(range(N_CORES)))
    return np.concatenate([res.results[i]["out"] for i in range(N_CORES)], axis=0)
